# revision 15
# baseline (speedup 1.0000x reference)
"""Trainium2 Bass kernel for the EnhancedBalSCL contrastive loss.

Full inputs in, full (scalar) output out. Internally data-parallel over the
batch dim across 8 NeuronCores; each core owns 512 rows of the batch and
produces per-row denominator partials; the host assembles the per-sample
losses (log, positive term, mean).

Math reformulation (validated to ~2.6e-4 vs the jax reference):
  w[k] = 1/(counts[t_k]+1), v[j] = 1/(counts[j]+1)
  denom[i] = sum_k exp(10*raw[i,k]) * w[k] + sum_j exp(10*rawc[i,j]) * v[j]
  P[i]     = sum_d F8[i,d] * U8[d, t_i],  U8 = fp8(H + C), H = class sums
  per_sample[i] = log(denom[i] + corr[i]) - (P[i] - diag8[i]) * 10 / counts[t_i]
where raw = F8 F8^T, rawc = F8 C8^T (fp8 operands, f32 accumulation), and
corr replaces the device's diagonal denominator term with the reference-grade
w_i*exp(10*||F_i||^2) (host-computed, exact cancellation of the device path).
The O(B*B*D) and O(B*C*D) exp-sums run on device; the O(B*D) positive term
P runs on the host during input prep.

Device mapping per core (512 rows = 4 row-tiles of 128; 20 column blocks of
1024: 4 singles for n0, 2048-wide supertiles (n1,n2) and (n3|centers) for
m0..m2, and n3/centers singles for m3 to shorten the serial tail):
  PE  : raw/rawc blocks fp8 DoubleRow (4 super-K tiles of 256), warm-up
        matmuls to start the p-state ramp during the DMA prologue.
  ACT : exp(10*x) per (super)tile — the bottleneck engine (~21us busy); the
        final log runs on the host, so no Exp->Ln table reload.
  DVE : scalar_tensor_tensor (exp_bf16 * w_bcast) weighted row-sums.
        (gpsimd/Pool cannot run DVE ALU ops on TRN2 — DMA only.)
Three blocks (n0 m0/m2, centers m3) instead fold ln(w)/10 into PSUM via a
ones-stationary bias matmul and let ACT accumulate exp in place (f32),
trading spare PE cycles for DVE time; the centers-m3 fold also removes the
last STT from the critical tail.

DMA rides three queues (sync/scalar/gpsimd); per-chunk ordering keeps the
rhs blocks just ahead of compute (each DMA costs ~1.6us latency beyond its
queue-busy time, so chunks are ordered by first-use).

Device output is [128, 13] f32 per core: denominator partials in cols
{3m, 3m+1, 3m+2} for row-tiles m<3 and {9, 10, 11, 12} for m=3.
"""

import numpy as np
import ml_dtypes

_B, _D, _C, _M = 4096, 1024, 1000, 8
_BL = _B // _M            # 512 rows per core
_RT = _BL // 128          # 4 row tiles per core
_JT = _D // 256           # 4 super-K tiles (fp8 DoubleRow path)
_NBW = 1024               # column block width
_NB = _B // _NBW          # 4 batch column blocks
_CP = 1024                # padded class dim
_W = _NB * _NBW + _CP     # w_bcast width (batch cols + padded center cols)
_SCALE = 10.0             # 1/tau

_CACHE = {}


def _build_nc(reps=1):
    # reps>1 wraps the compute schedule in a hardware loop (timing builds
    # only; the body is idempotent so results are unchanged)
    import concourse.bass as bass
    import concourse.mybir as mybir
    from concourse import bacc, tile
    from contextlib import ExitStack

    f32 = mybir.dt.float32
    bf16 = mybir.dt.bfloat16
    fp8 = mybir.dt.float8e4
    DR = mybir.MatmulPerfMode.DoubleRow
    AF = mybir.ActivationFunctionType
    OP = mybir.AluOpType

    nc = bacc.Bacc("TRN2", target_bir_lowering=False, debug=False,
                   num_devices=_M)
    f8_d = nc.declare_dram_parameter("ft8", [_NB, _JT, 2, 128, _NBW], fp8, isOutput=False)
    l8_d = nc.declare_dram_parameter("fl8", [_JT, 2, 128, _BL], fp8, isOutput=False)
    rc_d = nc.declare_dram_parameter("rc8", [_JT, 2, 128, _CP], fp8, isOutput=False)
    wv_d = nc.declare_dram_parameter("wv", [128, _W], bf16, isOutput=False)
    ln_d = nc.declare_dram_parameter("lnrow", [1, 2 * _NBW], bf16, isOutput=False)
    o1_d = nc.declare_dram_parameter("ones1", [1, 128], bf16, isOutput=False)
    out_d = nc.declare_dram_parameter("out", [128, 13], f32, isOutput=True)

    with tile.TileContext(nc) as tc, ExitStack() as ctx:
        consts = ctx.enter_context(tc.tile_pool(name="consts", bufs=1))
        psum = ctx.enter_context(tc.tile_pool(name="psum", bufs=1, space="PSUM"))
        xps = ctx.enter_context(tc.tile_pool(name="xps", bufs=4))
        scs = ctx.enter_context(tc.tile_pool(name="scs", bufs=3))

        # --- persistent SBUF residents -------------------------------------
        fl8 = consts.tile([128, _JT * 2 * _BL], fp8, tag="fl8")
        ft8 = [consts.tile([128, _JT * 2 * _NBW], fp8, tag=f"ft8_{n}", name=f"ft8_{n}")
               for n in range(_NB)]
        wv = consts.tile([128, _W], bf16, tag="wv")
        rc8t = consts.tile([128, _JT * 2 * _CP], fp8, tag="rct8")
        lnrow = consts.tile([1, 2 * _NBW], bf16, tag="lnrow")
        ones1 = consts.tile([1, 128], bf16, tag="ones1")
        outt = consts.tile([128, 13], f32, tag="outt")
        wz = consts.tile([128, 512], bf16, tag="wz")

        def ft8_chunk(n, j, eng):
            eng.dma_start(
                ft8[n][:, j * 2 * _NBW:(j + 1) * 2 * _NBW].rearrange(
                    "p (i c) -> p i c", i=2),
                f8_d[n, j].rearrange("i p c -> p i c"))

        def fl8_chunk(j, eng):
            eng.dma_start(
                fl8[:, j * 2 * _BL:(j + 1) * 2 * _BL].rearrange(
                    "p (i c) -> p i c", i=2),
                l8_d[j].rearrange("i p c -> p i c"))

        def rc8_chunk(j, eng):
            eng.dma_start(
                rc8t[:, j * 2 * _CP:(j + 1) * 2 * _CP].rearrange(
                    "p (i c) -> p i c", i=2),
                rc_d[j].rearrange("i p c -> p i c"))

        # sync queue: first-block chunks, then later rhs blocks in use order
        fl8_chunk(0, nc.sync)
        ft8_chunk(0, 0, nc.sync)
        fl8_chunk(1, nc.sync)
        ft8_chunk(0, 1, nc.sync)
        for j in range(_JT):
            ft8_chunk(1, j, nc.sync)
        ft8_chunk(2, 2, nc.sync)
        ft8_chunk(2, 3, nc.sync)
        for j in range(_JT):
            ft8_chunk(3, j, nc.sync)
        rc8_chunk(2, nc.sync)
        nc.sync.dma_start(ones1[:], o1_d[:])
        nc.sync.dma_start(lnrow[:], ln_d[:])

        # gpsimd queue: rest of the first block, wv chunks, ft8_2 j0/j1
        # (PE consumes ft8_2 j0 first, so the early queue carries it),
        # then the remaining centers chunks
        fl8_chunk(2, nc.gpsimd)
        fl8_chunk(3, nc.gpsimd)
        ft8_chunk(0, 2, nc.gpsimd)
        ft8_chunk(0, 3, nc.gpsimd)
        nc.gpsimd.dma_start(wv[:, 0:_NBW], wv_d[:, 0:_NBW])
        ft8_chunk(2, 0, nc.gpsimd)
        ft8_chunk(2, 1, nc.gpsimd)
        for s in range(1, _W // _NBW):
            nc.gpsimd.dma_start(wv[:, s * _NBW:(s + 1) * _NBW],
                                wv_d[:, s * _NBW:(s + 1) * _NBW])
        rc8_chunk(3, nc.gpsimd)
        rc8_chunk(0, nc.gpsimd)
        rc8_chunk(1, nc.gpsimd)

        # slice helpers
        lhs8 = [[fl8[:, j * 2 * _BL:(j + 1) * 2 * _BL]
                 .rearrange("p (i c) -> p i c", i=2)[:, :, m * 128:(m + 1) * 128]
                 for j in range(_JT)] for m in range(_RT)]

        nc.vector.memset(wz[:], 0.0)

        def blk():
            return psum.tile([128, 2048], f32, tag="blk", bufs=2, name="ps")

        def mm_block(ps, off, m, rhs_tile, fold=None):
            # one [128,1024] raw block at column offset off of the psum tile;
            # fold=0/1024 adds the ln-weight bias row (lnrow cols fold:+1024)
            # via a ones-stationary matmul so ACT's exp(10*x) weights in place
            for j in range(_JT):
                rj = rhs_tile[:, j * 2 * _NBW:(j + 1) * 2 * _NBW].rearrange(
                    "p (i c) -> p i c", i=2)
                for h in (0, 1):
                    o = off + h * 512
                    nc.tensor.matmul(ps[:, o:o + 512], lhs8[m][j],
                                     rj[:, :, h * 512:(h + 1) * 512],
                                     start=(j == 0),
                                     stop=(j == _JT - 1 and fold is None),
                                     perf_mode=DR)
            if fold is not None:
                for h in (0, 1):
                    s = fold + h * 512
                    nc.tensor.matmul(ps[:, off + h * 512:off + h * 512 + 512],
                                     ones1[:], lnrow[0:1, s:s + 512],
                                     start=False, stop=True)

        def wsum(ps, width, wlo, acc_col):
            # exp -> bf16 SBUF -> weighted row-sum on DVE
            xp = xps.tile([128, 2048], bf16, tag="xp", bufs=4, name="xp")
            nc.scalar.activation(xp[:, :width], ps[:, :width], AF.Exp,
                                 scale=_SCALE)
            sc = scs.tile([128, 2048], bf16, tag="sc", bufs=3, name="sc")
            nc.vector.scalar_tensor_tensor(
                out=sc[:, :width], in0=xp[:, :width], scalar=1.0,
                in1=wv[:, wlo:wlo + width],
                op0=OP.mult, op1=OP.mult,
                accum_out=outt[:, acc_col:acc_col + 1])

        def wfold(ps, width, acc_col):
            # bias already folded into PSUM: exp in place with f32 accum
            nc.scalar.activation(ps[:, :width], ps[:, :width], AF.Exp,
                                 scale=_SCALE,
                                 accum_out=outt[:, acc_col:acc_col + 1])

        # --- main schedule --------------------------------------------------
        def body(_i=None):
            # warm-up matmuls: start the PE p-state ramp during the prologue
            wps = blk()
            for _ in range(5):
                nc.tensor.matmul(wps[0:2, 0:512], wz[:, 0:2], wz[:, :],
                                 start=True, stop=True)
            # n0 singles
            for m in range(_RT):
                ps = blk()
                mm_block(ps, 0, m, ft8[0])
                wsum(ps, 1024, 0, 3 * m if m < 3 else 9)
            # (n1, n2) supertiles
            for m in range(_RT):
                ps = blk()
                mm_block(ps, 0, m, ft8[1])
                mm_block(ps, 1024, m, ft8[2])
                wsum(ps, 2048, 1024, 3 * m + 1 if m < 3 else 10)
            # (n3 | centers) supertiles for m0..m2; singles for m3
            for m in range(_RT - 1):
                ps = blk()
                mm_block(ps, 0, m, ft8[3])
                mm_block(ps, 1024, m, rc8t)
                wsum(ps, 2048, 3072, 3 * m + 2)
            # n3/centers m3 PE-folded: no STTs on the critical tail
            ps = blk()
            mm_block(ps, 0, 3, ft8[3], fold=0)
            wfold(ps, 1024, 11)
            ps = blk()
            mm_block(ps, 0, 3, rc8t, fold=1024)
            wfold(ps, 1024, 12)

        if reps == 1:
            body()
        else:
            # timing builds: hint the back-edge target to avoid an I$-miss
            # refetch per iteration (PE body exceeds one 256-instr IRAM block)
            with tc.For_i(0, reps, 1,
                          hint_engines=(mybir.EngineType.PE,)) as i:
                body(i)

        nc.sync.dma_start(out_d[:], outt[:])

    nc.compile()
    return nc


def _get_nc():
    if "nc" not in _CACHE:
        _CACHE["nc"] = _build_nc()
    return _CACHE["nc"]


def _prep_inputs(centers, features, targets):
    bf16 = ml_dtypes.bfloat16
    fp8 = ml_dtypes.float8_e4m3
    F = np.ascontiguousarray(features, dtype=np.float32)      # [B, D]
    Cen = np.ascontiguousarray(centers, dtype=np.float32)     # [C, D]
    t = np.asarray(targets).astype(np.int64).ravel()          # [B]

    counts = np.bincount(t, minlength=_C).astype(np.float32)  # [C]
    w = (1.0 / (counts[t] + 1.0)).astype(np.float32)          # [B]
    v = (1.0 / (counts + 1.0)).astype(np.float32)             # [C]
    H = np.zeros((_C, _D), dtype=np.float32)
    np.add.at(H, t, F)                                        # class sums

    F8 = F.astype(fp8)                                        # fp8 features
    F8f = F8.astype(np.float32)
    FT8 = np.ascontiguousarray(F8.T)                          # [D, B] fp8
    # fp8 rhs chunks [n][j, i, p, c]: k = j*256 + i*128 + p
    ft8 = np.ascontiguousarray(
        FT8.reshape(_JT, 2, 128, _NB, _NBW).transpose(3, 0, 1, 2, 4))
    CT8 = np.zeros((_D, _CP), dtype=fp8)
    CT8[:, :_C] = Cen.astype(fp8).T
    rc8 = np.ascontiguousarray(CT8.reshape(_JT, 2, 128, _CP))

    wb = w.astype(bf16)
    wv_row = np.zeros(_W, dtype=bf16)
    wv_row[:_B] = wb
    wv_row[_B:_B + _C] = v.astype(bf16)
    wv_full = np.ascontiguousarray(np.broadcast_to(wv_row, (128, _W)))

    # bias rows for the PE-folded m3 blocks: ln(w)/10 for batch cols
    # 3072:4096 and ln(v)/10 for centers (padded -3 so exp(10*0-30) ~ 0)
    lnrow = np.full(2 * _NBW, -3.0, dtype=bf16)
    lnrow[:_NBW] = (np.log(w[3 * _NBW:]) / np.float32(_SCALE)).astype(bf16)
    lnrow[_NBW:_NBW + _C] = (np.log(v) / np.float32(_SCALE)).astype(bf16)

    # host-side finals: positive term P (O(B*D)) and the exact cancellation
    # of the device's diagonal denominator term.  The diagonal of row i sits
    # in batch block n = i//1024, which is the n0 block of cores 2n/2n+1:
    # row-tiles m0/m2 of those cores take the PE-folded path (f32
    # exp(10*diag8 + ln w_bf16)); all other rows take the bf16 STT path.
    U8f = (H + Cen).astype(fp8).astype(np.float32)            # [C, D]
    P = np.einsum("bd,bd->b", F8f, U8f[t, :], dtype=np.float32)
    diag8 = np.einsum("bd,bd->b", F8f, F8f, dtype=np.float32)
    lnw_b = (np.log(w) / np.float32(_SCALE)).astype(bf16).astype(np.float32)
    dev_stt = (np.exp(np.float32(_SCALE) * diag8).astype(bf16).astype(np.float32)
               * wb.astype(np.float32)).astype(bf16).astype(np.float32)
    dev_fold = np.exp(np.float32(_SCALE) * (diag8 + lnw_b))
    i_all = np.arange(_B)
    m_all = (i_all % _BL) // 128
    folded = (i_all >= 3 * _NBW) & (m_all == 3)  # n3-block diag, m3 rows
    dev_diag = np.where(folded, dev_fold, dev_stt)
    q = (F * F).sum(axis=1)                                   # reference-grade
    corr = w * np.exp(np.float32(_SCALE) * q) - dev_diag
    pos = (P - diag8) * (np.float32(_SCALE) / counts[t])      # [B]

    def col(x_loc):  # [512] -> [128, RT] with (p, m) = x[m*128+p]
        return np.ascontiguousarray(x_loc.reshape(_RT, 128).T)

    in_maps = []
    host = []
    for c in range(_M):
        R = c * _BL
        fl8 = np.ascontiguousarray(FT8[:, R:R + _BL]).reshape(_JT, 2, 128, _BL)
        in_maps.append({
            "ft8": ft8, "fl8": fl8, "rc8": rc8, "wv": wv_full,
            "lnrow": lnrow.reshape(1, 2 * _NBW),
            "ones1": np.ones((1, 128), dtype=bf16),
        })
        host.append({
            "corr": col(corr[R:R + _BL]),
            "pos": col(pos[R:R + _BL]),
        })
    _CACHE["host"] = host
    return in_maps


def _assemble(results):
    # device out [128, 13] per core: den partials in cols {3m, 3m+1, 3m+2}
    # for m<3 and {9, 10, 11, 12} for m=3
    host = _CACHE["host"]
    total = 0.0
    for c in range(_M):
        o = np.asarray(results[c]["out"], dtype=np.float32)
        h = host[c]
        den = np.stack(
            [o[:, 3 * m] + o[:, 3 * m + 1] + o[:, 3 * m + 2] if m < 3
             else o[:, 9] + o[:, 10] + o[:, 11] + o[:, 12]
             for m in range(_RT)], axis=1)
        den = den + h["corr"]                                  # [128, RT]
        per = np.log(den) - h["pos"]
        total += float(per.sum())
    return np.float32(total / _B)


def _run(inputs, trace=False, **trace_kwargs):
    from concourse.bass_utils import run_bass_kernel_spmd
    nc = _get_nc()
    in_maps = _prep_inputs(**inputs)
    res = run_bass_kernel_spmd(nc, in_maps, core_ids=list(range(_M)),
                               trace=trace, **trace_kwargs)
    return _assemble(res.results), res


def kernel(centers, features, targets):
    out, _ = _run({"centers": centers, "features": features, "targets": targets})
    return out


# revision 16
# speedup vs baseline: 1.2249x; 1.2249x over previous
"""Trainium2 Bass kernel for the EnhancedBalSCL contrastive loss.

Full inputs in, full (scalar) output out. Internally data-parallel over the
batch dim across 8 NeuronCores; each core owns 512 rows of the batch and
produces per-row denominator partials; the host assembles the per-sample
losses (log, positive term, mean).

Math reformulation (validated to ~2.6e-4 vs the jax reference):
  w[k] = 1/(counts[t_k]+1), v[j] = 1/(counts[j]+1)
  denom[i] = sum_k exp(10*raw[i,k]) * w[k] + sum_j exp(10*rawc[i,j]) * v[j]
  P[i]     = sum_d F8[i,d] * U8[d, t_i],  U8 = fp8(H + C), H = class sums
  per_sample[i] = log(denom[i] + corr[i]) - (P[i] - diag8[i]) * 10 / counts[t_i]
where raw = F8 F8^T, rawc = F8 C8^T (fp8 operands, f32 accumulation), and
corr replaces the device's diagonal denominator term with the reference-grade
w_i*exp(10*||F_i||^2) (host-computed, exact cancellation of the device path).
The O(B*B*D) and O(B*C*D) exp-sums run on device; the O(B*D) positive term
P runs on the host during input prep.

Device mapping per core (512 rows = 4 row-tiles of 128; 20 column blocks of
1024: 4 singles for n0, 2048-wide supertiles (n1,n2) and (n3|centers) for
m0..m2, and n3/centers singles for m3 to shorten the serial tail):
  PE  : raw/rawc blocks fp8 DoubleRow (4 super-K tiles of 256) — the
        bottleneck engine on hardware (~42us: the fp8 roofline for this
        work split is 34.1us/core, plus ~50ns/instruction overhead).
  ACT : exp(10*x) per (super)tile (~21us busy, hidden under PE); the
        final log runs on the host, so no Exp->Ln table reload.
  DVE : scalar_tensor_tensor (exp_bf16 * w_bcast) weighted row-sums.
        (gpsimd/Pool cannot run DVE ALU ops on TRN2 — DMA only.)
The two m=3 tail blocks instead fold ln(w)/10 into PSUM via a
ones-stationary bias matmul and let ACT accumulate exp in place (f32),
removing the last STTs from the critical tail.

DMA rides three queues (sync/scalar/gpsimd); per-chunk ordering keeps the
rhs blocks just ahead of compute (each DMA costs ~1.6us latency beyond its
queue-busy time, so chunks are ordered by first-use).

Device output is [128, 13] f32 per core: denominator partials in cols
{3m, 3m+1, 3m+2} for row-tiles m<3 and {9, 10, 11, 12} for m=3.
"""

import numpy as np
import ml_dtypes

_B, _D, _C, _M = 4096, 1024, 1000, 8
_BL = _B // _M            # 512 rows per core
_RT = _BL // 128          # 4 row tiles per core
_JT = _D // 256           # 4 super-K tiles (fp8 DoubleRow path)
_NBW = 1024               # column block width
_NB = _B // _NBW          # 4 batch column blocks
_CP = 1024                # padded class dim
_W = _NB * _NBW + _CP     # w_bcast width (batch cols + padded center cols)
_SCALE = 10.0             # 1/tau

_CACHE = {}


def _build_nc(reps=1):
    # reps>1 wraps the compute schedule in a hardware loop (timing builds
    # only; the body is idempotent so results are unchanged)
    import concourse.bass as bass
    import concourse.mybir as mybir
    from concourse import bacc, tile
    from contextlib import ExitStack

    f32 = mybir.dt.float32
    bf16 = mybir.dt.bfloat16
    fp8 = mybir.dt.float8e4
    DR = mybir.MatmulPerfMode.DoubleRow
    AF = mybir.ActivationFunctionType
    OP = mybir.AluOpType

    nc = bacc.Bacc("TRN2", target_bir_lowering=False, debug=False,
                   num_devices=_M)
    f8_d = nc.declare_dram_parameter("ft8", [_NB, _JT, 2, 128, _NBW], fp8, isOutput=False)
    l8_d = nc.declare_dram_parameter("fl8", [_JT, 2, 128, _BL], fp8, isOutput=False)
    rc_d = nc.declare_dram_parameter("rc8", [_JT, 2, 128, _CP], fp8, isOutput=False)
    wv_d = nc.declare_dram_parameter("wv", [128, _W], bf16, isOutput=False)
    ln_d = nc.declare_dram_parameter("lnrow", [1, 2 * _NBW], bf16, isOutput=False)
    o1_d = nc.declare_dram_parameter("ones1", [1, 128], bf16, isOutput=False)
    out_d = nc.declare_dram_parameter("out", [128, 13], f32, isOutput=True)

    with tile.TileContext(nc) as tc, ExitStack() as ctx:
        consts = ctx.enter_context(tc.tile_pool(name="consts", bufs=1))
        psum = ctx.enter_context(tc.tile_pool(name="psum", bufs=1, space="PSUM"))
        xps = ctx.enter_context(tc.tile_pool(name="xps", bufs=4))
        scs = ctx.enter_context(tc.tile_pool(name="scs", bufs=3))

        # --- persistent SBUF residents -------------------------------------
        fl8 = consts.tile([128, _JT * 2 * _BL], fp8, tag="fl8")
        ft8 = [consts.tile([128, _JT * 2 * _NBW], fp8, tag=f"ft8_{n}", name=f"ft8_{n}")
               for n in range(_NB)]
        wv = consts.tile([128, _W], bf16, tag="wv")
        rc8t = consts.tile([128, _JT * 2 * _CP], fp8, tag="rct8")
        lnrow = consts.tile([1, 2 * _NBW], bf16, tag="lnrow")
        ones1 = consts.tile([1, 128], bf16, tag="ones1")
        outt = consts.tile([128, 13], f32, tag="outt")

        def ft8_chunk(n, j, eng):
            eng.dma_start(
                ft8[n][:, j * 2 * _NBW:(j + 1) * 2 * _NBW].rearrange(
                    "p (i c) -> p i c", i=2),
                f8_d[n, j].rearrange("i p c -> p i c"))

        def fl8_chunk(j, eng):
            eng.dma_start(
                fl8[:, j * 2 * _BL:(j + 1) * 2 * _BL].rearrange(
                    "p (i c) -> p i c", i=2),
                l8_d[j].rearrange("i p c -> p i c"))

        def rc8_chunk(j, eng):
            eng.dma_start(
                rc8t[:, j * 2 * _CP:(j + 1) * 2 * _CP].rearrange(
                    "p (i c) -> p i c", i=2),
                rc_d[j].rearrange("i p c -> p i c"))

        # sync queue: first-block chunks, then later rhs blocks in use order
        fl8_chunk(0, nc.sync)
        ft8_chunk(0, 0, nc.sync)
        fl8_chunk(1, nc.sync)
        ft8_chunk(0, 1, nc.sync)
        for j in range(_JT):
            ft8_chunk(1, j, nc.sync)
        ft8_chunk(2, 2, nc.sync)
        ft8_chunk(2, 3, nc.sync)
        for j in range(_JT):
            ft8_chunk(3, j, nc.sync)
        rc8_chunk(2, nc.sync)
        nc.sync.dma_start(ones1[:], o1_d[:])
        nc.sync.dma_start(lnrow[:], ln_d[:])

        # gpsimd queue: rest of the first block, wv chunks, ft8_2 j0/j1
        # (PE consumes ft8_2 j0 first, so the early queue carries it),
        # then the remaining centers chunks
        fl8_chunk(2, nc.gpsimd)
        fl8_chunk(3, nc.gpsimd)
        ft8_chunk(0, 2, nc.gpsimd)
        ft8_chunk(0, 3, nc.gpsimd)
        nc.gpsimd.dma_start(wv[:, 0:_NBW], wv_d[:, 0:_NBW])
        ft8_chunk(2, 0, nc.gpsimd)
        ft8_chunk(2, 1, nc.gpsimd)
        for s in range(1, _W // _NBW):
            nc.gpsimd.dma_start(wv[:, s * _NBW:(s + 1) * _NBW],
                                wv_d[:, s * _NBW:(s + 1) * _NBW])
        rc8_chunk(3, nc.gpsimd)
        rc8_chunk(0, nc.gpsimd)
        rc8_chunk(1, nc.gpsimd)

        # slice helpers
        lhs8 = [[fl8[:, j * 2 * _BL:(j + 1) * 2 * _BL]
                 .rearrange("p (i c) -> p i c", i=2)[:, :, m * 128:(m + 1) * 128]
                 for j in range(_JT)] for m in range(_RT)]

        def blk():
            return psum.tile([128, 2048], f32, tag="blk", bufs=2, name="ps")

        def mm_block(ps, off, m, rhs_tile, fold=None):
            # one [128,1024] raw block at column offset off of the psum tile;
            # fold=0/1024 adds the ln-weight bias row (lnrow cols fold:+1024)
            # via a ones-stationary matmul so ACT's exp(10*x) weights in place
            for j in range(_JT):
                rj = rhs_tile[:, j * 2 * _NBW:(j + 1) * 2 * _NBW].rearrange(
                    "p (i c) -> p i c", i=2)
                for h in (0, 1):
                    o = off + h * 512
                    nc.tensor.matmul(ps[:, o:o + 512], lhs8[m][j],
                                     rj[:, :, h * 512:(h + 1) * 512],
                                     start=(j == 0),
                                     stop=(j == _JT - 1 and fold is None),
                                     perf_mode=DR)
            if fold is not None:
                for h in (0, 1):
                    s = fold + h * 512
                    nc.tensor.matmul(ps[:, off + h * 512:off + h * 512 + 512],
                                     ones1[:], lnrow[0:1, s:s + 512],
                                     start=False, stop=True)

        def wsum(ps, width, wlo, acc_col):
            # exp -> bf16 SBUF -> weighted row-sum on DVE
            xp = xps.tile([128, 2048], bf16, tag="xp", bufs=4, name="xp")
            nc.scalar.activation(xp[:, :width], ps[:, :width], AF.Exp,
                                 scale=_SCALE)
            sc = scs.tile([128, 2048], bf16, tag="sc", bufs=3, name="sc")
            nc.vector.scalar_tensor_tensor(
                out=sc[:, :width], in0=xp[:, :width], scalar=1.0,
                in1=wv[:, wlo:wlo + width],
                op0=OP.mult, op1=OP.mult,
                accum_out=outt[:, acc_col:acc_col + 1])

        def wfold(ps, width, acc_col):
            # bias already folded into PSUM: exp in place with f32 accum
            nc.scalar.activation(ps[:, :width], ps[:, :width], AF.Exp,
                                 scale=_SCALE,
                                 accum_out=outt[:, acc_col:acc_col + 1])

        # --- main schedule --------------------------------------------------
        def body(_i=None):
            # n0 singles
            for m in range(_RT):
                ps = blk()
                mm_block(ps, 0, m, ft8[0])
                wsum(ps, 1024, 0, 3 * m if m < 3 else 9)
            # (n1, n2) supertiles
            for m in range(_RT):
                ps = blk()
                mm_block(ps, 0, m, ft8[1])
                mm_block(ps, 1024, m, ft8[2])
                wsum(ps, 2048, 1024, 3 * m + 1 if m < 3 else 10)
            # (n3 | centers) supertiles for m0..m2; singles for m3
            for m in range(_RT - 1):
                ps = blk()
                mm_block(ps, 0, m, ft8[3])
                mm_block(ps, 1024, m, rc8t)
                wsum(ps, 2048, 3072, 3 * m + 2)
            # n3/centers m3 PE-folded: no STTs on the critical tail
            ps = blk()
            mm_block(ps, 0, 3, ft8[3], fold=0)
            wfold(ps, 1024, 11)
            ps = blk()
            mm_block(ps, 0, 3, rc8t, fold=1024)
            wfold(ps, 1024, 12)

        if reps == 1:
            body()
        else:
            # timing builds: hint the back-edge target to avoid an I$-miss
            # refetch per iteration (PE body exceeds one 256-instr IRAM block)
            with tc.For_i(0, reps, 1,
                          hint_engines=(mybir.EngineType.PE,)) as i:
                body(i)

        nc.sync.dma_start(out_d[:], outt[:])

    nc.compile()
    return nc


def _get_nc():
    if "nc" not in _CACHE:
        _CACHE["nc"] = _build_nc()
    return _CACHE["nc"]


def _prep_inputs(centers, features, targets):
    bf16 = ml_dtypes.bfloat16
    fp8 = ml_dtypes.float8_e4m3
    F = np.ascontiguousarray(features, dtype=np.float32)      # [B, D]
    Cen = np.ascontiguousarray(centers, dtype=np.float32)     # [C, D]
    t = np.asarray(targets).astype(np.int64).ravel()          # [B]

    counts = np.bincount(t, minlength=_C).astype(np.float32)  # [C]
    w = (1.0 / (counts[t] + 1.0)).astype(np.float32)          # [B]
    v = (1.0 / (counts + 1.0)).astype(np.float32)             # [C]
    H = np.zeros((_C, _D), dtype=np.float32)
    np.add.at(H, t, F)                                        # class sums

    F8 = F.astype(fp8)                                        # fp8 features
    F8f = F8.astype(np.float32)
    FT8 = np.ascontiguousarray(F8.T)                          # [D, B] fp8
    # fp8 rhs chunks [n][j, i, p, c]: k = j*256 + i*128 + p
    ft8 = np.ascontiguousarray(
        FT8.reshape(_JT, 2, 128, _NB, _NBW).transpose(3, 0, 1, 2, 4))
    CT8 = np.zeros((_D, _CP), dtype=fp8)
    CT8[:, :_C] = Cen.astype(fp8).T
    rc8 = np.ascontiguousarray(CT8.reshape(_JT, 2, 128, _CP))

    wb = w.astype(bf16)
    wv_row = np.zeros(_W, dtype=bf16)
    wv_row[:_B] = wb
    wv_row[_B:_B + _C] = v.astype(bf16)
    wv_full = np.ascontiguousarray(np.broadcast_to(wv_row, (128, _W)))

    # bias rows for the PE-folded m3 blocks: ln(w)/10 for batch cols
    # 3072:4096 and ln(v)/10 for centers (padded -3 so exp(10*0-30) ~ 0)
    lnrow = np.full(2 * _NBW, -3.0, dtype=bf16)
    lnrow[:_NBW] = (np.log(w[3 * _NBW:]) / np.float32(_SCALE)).astype(bf16)
    lnrow[_NBW:_NBW + _C] = (np.log(v) / np.float32(_SCALE)).astype(bf16)

    # host-side finals: positive term P (O(B*D)) and the exact cancellation
    # of the device's diagonal denominator term.  The diagonal of row i sits
    # in batch block n = i//1024, which is the n0 block of cores 2n/2n+1:
    # row-tiles m0/m2 of those cores take the PE-folded path (f32
    # exp(10*diag8 + ln w_bf16)); all other rows take the bf16 STT path.
    U8f = (H + Cen).astype(fp8).astype(np.float32)            # [C, D]
    P = np.einsum("bd,bd->b", F8f, U8f[t, :], dtype=np.float32)
    diag8 = np.einsum("bd,bd->b", F8f, F8f, dtype=np.float32)
    lnw_b = (np.log(w) / np.float32(_SCALE)).astype(bf16).astype(np.float32)
    dev_stt = (np.exp(np.float32(_SCALE) * diag8).astype(bf16).astype(np.float32)
               * wb.astype(np.float32)).astype(bf16).astype(np.float32)
    dev_fold = np.exp(np.float32(_SCALE) * (diag8 + lnw_b))
    i_all = np.arange(_B)
    m_all = (i_all % _BL) // 128
    folded = (i_all >= 3 * _NBW) & (m_all == 3)  # n3-block diag, m3 rows
    dev_diag = np.where(folded, dev_fold, dev_stt)
    q = (F * F).sum(axis=1)                                   # reference-grade
    corr = w * np.exp(np.float32(_SCALE) * q) - dev_diag
    pos = (P - diag8) * (np.float32(_SCALE) / counts[t])      # [B]

    def col(x_loc):  # [512] -> [128, RT] with (p, m) = x[m*128+p]
        return np.ascontiguousarray(x_loc.reshape(_RT, 128).T)

    in_maps = []
    host = []
    for c in range(_M):
        R = c * _BL
        fl8 = np.ascontiguousarray(FT8[:, R:R + _BL]).reshape(_JT, 2, 128, _BL)
        in_maps.append({
            "ft8": ft8, "fl8": fl8, "rc8": rc8, "wv": wv_full,
            "lnrow": lnrow.reshape(1, 2 * _NBW),
            "ones1": np.ones((1, 128), dtype=bf16),
        })
        host.append({
            "corr": col(corr[R:R + _BL]),
            "pos": col(pos[R:R + _BL]),
        })
    _CACHE["host"] = host
    return in_maps


def _assemble(results):
    # device out [128, 13] per core: den partials in cols {3m, 3m+1, 3m+2}
    # for m<3 and {9, 10, 11, 12} for m=3
    host = _CACHE["host"]
    total = 0.0
    for c in range(_M):
        o = np.asarray(results[c]["out"], dtype=np.float32)
        h = host[c]
        den = np.stack(
            [o[:, 3 * m] + o[:, 3 * m + 1] + o[:, 3 * m + 2] if m < 3
             else o[:, 9] + o[:, 10] + o[:, 11] + o[:, 12]
             for m in range(_RT)], axis=1)
        den = den + h["corr"]                                  # [128, RT]
        per = np.log(den) - h["pos"]
        total += float(per.sum())
    return np.float32(total / _B)


def _run(inputs, trace=False, **trace_kwargs):
    from concourse.bass_utils import run_bass_kernel_spmd
    nc = _get_nc()
    in_maps = _prep_inputs(**inputs)
    res = run_bass_kernel_spmd(nc, in_maps, core_ids=list(range(_M)),
                               trace=trace, **trace_kwargs)
    return _assemble(res.results), res


def kernel(centers, features, targets):
    out, _ = _run({"centers": centers, "features": features, "targets": targets})
    return out


# revision 17
# speedup vs baseline: 1.3310x; 1.0867x over previous
"""Symmetric-decomposition variant of the EnhancedBalSCL TRN2 kernel.

raw = F F^T is symmetric, so each unordered 512x512 block pair is computed
ONCE: core c computes its diagonal block plus off-diagonal blocks against
cores c+1, c+2, c+3 (mod 8), and cores 0..3 also compute the {c, c+4} pair
block (cores 4..7 run a zero-padded dummy there to keep the SPMD program
uniform).  Each off-diagonal block yields BOTH:
  - row-sums   sum_k w_k exp(10 raw[i,k])  (DVE STT, as before) -> denom_i
  - col-sums   sum_i w_i exp(10 raw[i,k])  (PE ones-matmul with the w column
    as stationary, accumulated over the 4 row-tiles in PSUM)       -> denom_k
The host scatters the col-sums into the other cores' denominators.  Batch
columns per core drop from 4096 to 2560 (2048 real on cores 4..7), cutting
PE matmul count from 172 to 128 and total DMA from 6.6MiB to ~4.4MiB.

Everything else (fp8 DoubleRow raw blocks, bf16 exp via ACT -> SBUF, host-side
P/log/assembly, exact diag-term cancellation) matches kernel.py.

Device outputs per core:
  out  [128, 16] f32: row-sum partials, col 4m+u, u in {diag, pairA, pairB,
       centers} for row-tile m
  out2 [4, 512]  f32: col-sums of the 4 off-diag blocks (q -> block c+1+q),
       read from PSUM partitions 0/32/64/96
"""

import numpy as np
import ml_dtypes

_B, _D, _C, _M = 4096, 1024, 1000, 8
_BL = _B // _M            # 512 rows per core
_RT = _BL // 128          # 4 row tiles per core
_JT = _D // 256           # 4 super-K tiles (fp8 DoubleRow path)
_XC = 2048                # off-diagonal rhs columns per core (4 blocks)
_CP = 1024                # padded class dim
_W = _BL + _XC + _CP      # per-core w row: own | off-blocks | centers
_SCALE = 10.0             # 1/tau

_CACHE = {}


def _build_nc(reps=1):
    import concourse.bass as bass
    import concourse.mybir as mybir
    from concourse import bacc, tile
    from contextlib import ExitStack

    f32 = mybir.dt.float32
    bf16 = mybir.dt.bfloat16
    fp8 = mybir.dt.float8e4
    DR = mybir.MatmulPerfMode.DoubleRow
    AF = mybir.ActivationFunctionType
    OP = mybir.AluOpType

    nc = bacc.Bacc("TRN2", target_bir_lowering=False, debug=False,
                   num_devices=_M)
    fs_d = nc.declare_dram_parameter("fts", [_JT, 2, 128, _XC], fp8, isOutput=False)
    l8_d = nc.declare_dram_parameter("fl8", [_JT, 2, 128, _BL], fp8, isOutput=False)
    rc_d = nc.declare_dram_parameter("rc8", [_JT, 2, 128, _CP], fp8, isOutput=False)
    wv_d = nc.declare_dram_parameter("wv", [128, _W], bf16, isOutput=False)
    wc_d = nc.declare_dram_parameter("wcol", [128, _RT], bf16, isOutput=False)
    out_d = nc.declare_dram_parameter("out", [128, 16], f32, isOutput=True)
    o2_d = nc.declare_dram_parameter("out2", [4, 512], f32, isOutput=True)

    with tile.TileContext(nc) as tc, ExitStack() as ctx:
        consts = ctx.enter_context(tc.tile_pool(name="consts", bufs=1))
        psum = ctx.enter_context(tc.tile_pool(name="psum", bufs=1, space="PSUM"))
        xps = ctx.enter_context(tc.tile_pool(name="xps", bufs=4))
        scs = ctx.enter_context(tc.tile_pool(name="scs", bufs=3))

        fl8 = consts.tile([128, _JT * 2 * _BL], fp8, tag="fl8")
        fts = consts.tile([128, _JT * 2 * _XC], fp8, tag="fts")
        wv = consts.tile([128, _W], bf16, tag="wv")
        rc8t = consts.tile([128, _JT * 2 * _CP], fp8, tag="rct8")
        wcol = consts.tile([128, _RT], bf16, tag="wcol")
        outt = consts.tile([128, 16], f32, tag="outt")
        o2s = consts.tile([128, 512], f32, tag="o2s")

        def fl8_chunk(j, eng):
            eng.dma_start(
                fl8[:, j * 2 * _BL:(j + 1) * 2 * _BL].rearrange(
                    "p (i c) -> p i c", i=2),
                l8_d[j].rearrange("i p c -> p i c"))

        def fts_chunk(j, g, eng):
            # column-group g (0: cols 0:1024, 1: cols 1024:2048) of k-tile j
            eng.dma_start(
                fts[:, j * 2 * _XC:(j + 1) * 2 * _XC]
                .rearrange("p (i c) -> p i c", i=2)[:, :, g * 1024:(g + 1) * 1024],
                fs_d[j, :, :, g * 1024:(g + 1) * 1024].rearrange("i p c -> p i c"))

        def rc8_chunk(j, eng):
            eng.dma_start(
                rc8t[:, j * 2 * _CP:(j + 1) * 2 * _CP].rearrange(
                    "p (i c) -> p i c", i=2),
                rc_d[j].rearrange("i p c -> p i c"))

        # sync queue: fl8 j0/j1 (diag gate), then fts groups in use order
        fl8_chunk(0, nc.sync)
        fl8_chunk(1, nc.sync)
        for j in range(_JT):
            fts_chunk(j, 0, nc.sync)
        for j in range(_JT):
            fts_chunk(j, 1, nc.sync)
        rc8_chunk(2, nc.sync)
        rc8_chunk(3, nc.sync)

        # gpsimd queue: fl8 j2/j3, the w row (diag slice first), wcol, rc8
        fl8_chunk(2, nc.gpsimd)
        fl8_chunk(3, nc.gpsimd)
        nc.gpsimd.dma_start(wv[:, 0:_BL], wv_d[:, 0:_BL])
        nc.gpsimd.dma_start(wcol[:], wc_d[:])
        for s in range(3):
            lo = _BL + s * 1024
            nc.gpsimd.dma_start(wv[:, lo:lo + 1024], wv_d[:, lo:lo + 1024])
        rc8_chunk(0, nc.gpsimd)
        rc8_chunk(1, nc.gpsimd)

        lhs8 = [[fl8[:, j * 2 * _BL:(j + 1) * 2 * _BL]
                 .rearrange("p (i c) -> p i c", i=2)[:, :, m * 128:(m + 1) * 128]
                 for j in range(_JT)] for m in range(_RT)]

        def blk():
            return psum.tile([128, 1024], f32, tag="blk", bufs=3, name="ps")

        def mm_cols(ps, off, m, src_tile, src_w, lo, width):
            # raw block [128, width] from columns lo:lo+width of src_tile
            for j in range(_JT):
                rj = src_tile[:, j * 2 * src_w:(j + 1) * 2 * src_w].rearrange(
                    "p (i c) -> p i c", i=2)
                for h in range(width // 512):
                    o = off + h * 512
                    s = lo + h * 512
                    nc.tensor.matmul(ps[:, o:o + 512], lhs8[m][j],
                                     rj[:, :, s:s + 512],
                                     start=(j == 0), stop=(j == _JT - 1),
                                     perf_mode=DR)

        def wsum(ps, width, wlo, acc_col):
            xp = xps.tile([128, 1024], bf16, tag="xp", bufs=4, name="xp")
            nc.scalar.activation(xp[:, :width], ps[:, :width], AF.Exp,
                                 scale=_SCALE)
            sc = scs.tile([128, 1024], bf16, tag="sc", bufs=3, name="sc")
            nc.vector.scalar_tensor_tensor(
                out=sc[:, :width], in0=xp[:, :width], scalar=1.0,
                in1=wv[:, wlo:wlo + width],
                op0=OP.mult, op1=OP.mult,
                accum_out=outt[:, acc_col:acc_col + 1])
            return xp

        def body(_i=None):
            cs = psum.tile([128, 512], f32, tag="cs", bufs=1, name="cs")
            cs2 = psum.tile([128, 512], f32, tag="cs2", bufs=1, name="cs2")

            deferred = []

            def drain(keep):
                # emit deferred col-sum matmuls lagged behind their block so
                # PE never waits on the ACT->SBUF exp ack latency
                while len(deferred) > keep:
                    deferred.pop(0)()

            def colsum_of(pair, m, xp):
                def emit():
                    for q in (2 * pair, 2 * pair + 1):
                        tgt = cs[32 * q:32 * q + 1, 0:512] if q < 3 \
                            else cs2[0:1, 0:512]
                        nc.tensor.matmul(
                            tgt, wcol[:, m:m + 1],
                            xp[:, (q % 2) * 512:(q % 2) * 512 + 512],
                            start=(m == 0), stop=(m == _RT - 1))
                return emit

            # diag unit: needs only fl8; starts the pipeline early
            for m in range(_RT):
                ps = blk()
                mm_cols(ps, 0, m, fl8, _BL, 0, 512)
                wsum(ps, 512, 0, 4 * m)
            # off-diagonal pair units; col-sums accumulate in cs, lagged 2
            for pair in (0, 1):
                for m in range(_RT):
                    ps = blk()
                    mm_cols(ps, 0, m, fts, _XC, pair * 1024, 1024)
                    drain(2)
                    xp = wsum(ps, 1024, _BL + pair * 1024, 4 * m + 1 + pair)
                    deferred.append(colsum_of(pair, m, xp))
            # centers (remaining col-sums drain between the center blocks)
            for m in range(_RT):
                ps = blk()
                mm_cols(ps, 0, m, rc8t, _CP, 0, 1024)
                drain(1 - m if m < 2 else 0)
                wsum(ps, 1024, _BL + _XC, 4 * m + 3)
            drain(0)
            # stage the finished col-sums to SBUF (PSUM cannot be DMA'd)
            for q in range(3):
                nc.scalar.activation(o2s[32 * q:32 * q + 1, :],
                                     cs[32 * q:32 * q + 1, :], AF.Copy)
            nc.scalar.activation(o2s[96:97, :], cs2[0:1, :], AF.Copy)

        if reps == 1:
            body()
        else:
            with tc.For_i(0, reps, 1,
                          hint_engines=(mybir.EngineType.PE,)) as i:
                body(i)

        nc.sync.dma_start(out_d[:], outt[:])
        nc.sync.dma_start(
            o2_d[0:3],
            o2s[0:96].rearrange("(a b) c -> a b c", b=32)[:, 0:1, :])
        nc.sync.dma_start(o2_d[3:4], o2s[96:97, :])

    nc.compile()
    return nc


def _get_nc():
    if "nc" not in _CACHE:
        _CACHE["nc"] = _build_nc()
    return _CACHE["nc"]


def _prep_inputs(centers, features, targets):
    bf16 = ml_dtypes.bfloat16
    fp8 = ml_dtypes.float8_e4m3
    F = np.ascontiguousarray(features, dtype=np.float32)      # [B, D]
    Cen = np.ascontiguousarray(centers, dtype=np.float32)     # [C, D]
    t = np.asarray(targets).astype(np.int64).ravel()          # [B]

    counts = np.bincount(t, minlength=_C).astype(np.float32)
    w = (1.0 / (counts[t] + 1.0)).astype(np.float32)
    v = (1.0 / (counts + 1.0)).astype(np.float32)
    H = np.zeros((_C, _D), dtype=np.float32)
    np.add.at(H, t, F)

    F8 = F.astype(fp8)
    F8f = F8.astype(np.float32)
    FT8 = np.ascontiguousarray(F8.T)                          # [D, B] fp8
    CT8 = np.zeros((_D, _CP), dtype=fp8)
    CT8[:, :_C] = Cen.astype(fp8).T
    rc8 = np.ascontiguousarray(CT8.reshape(_JT, 2, 128, _CP))

    wb = w.astype(bf16)
    vb16 = v.astype(bf16)

    # host-side finals: positive term P and the diag-term cancellation
    U8f = (H + Cen).astype(fp8).astype(np.float32)
    P = np.einsum("bd,bd->b", F8f, U8f[t, :], dtype=np.float32)
    diag8 = np.einsum("bd,bd->b", F8f, F8f, dtype=np.float32)
    dev_diag = (np.exp(np.float32(_SCALE) * diag8).astype(bf16).astype(np.float32)
                * wb.astype(np.float32)).astype(bf16).astype(np.float32)
    q = (F * F).sum(axis=1)
    corr = w * np.exp(np.float32(_SCALE) * q) - dev_diag
    pos = (P - diag8) * (np.float32(_SCALE) / counts[t])

    def col(x_loc):
        return np.ascontiguousarray(x_loc.reshape(_RT, 128).T)

    in_maps = []
    host = []
    for c in range(_M):
        R = c * _BL
        fl8 = np.ascontiguousarray(FT8[:, R:R + _BL]).reshape(_JT, 2, 128, _BL)
        # off-diagonal rhs: blocks c+1, c+2, c+3 and (c<4) c+4, else zeros
        blocks = [(c + 1) % _M, (c + 2) % _M, (c + 3) % _M]
        if c < 4:
            blocks.append(c + 4)
        cols = np.concatenate([np.arange(b * _BL, (b + 1) * _BL) for b in blocks])
        fts = np.zeros((_D, _XC), dtype=fp8)
        fts[:, :len(cols)] = FT8[:, cols]
        fts = np.ascontiguousarray(fts.reshape(_JT, 2, 128, _XC))
        wv_row = np.zeros(_W, dtype=bf16)
        wv_row[0:_BL] = wb[R:R + _BL]
        wv_row[_BL:_BL + len(cols)] = wb[cols]
        wv_row[_BL + _XC:_BL + _XC + _C] = vb16
        wv_full = np.ascontiguousarray(np.broadcast_to(wv_row, (128, _W)))
        in_maps.append({
            "fts": fts, "fl8": fl8, "rc8": rc8, "wv": wv_full,
            "wcol": col(wb[R:R + _BL].astype(np.float32)).astype(bf16),
        })
        host.append({
            "corr": col(corr[R:R + _BL]),
            "pos": col(pos[R:R + _BL]),
            "blocks": blocks,
        })
    _CACHE["host"] = host
    return in_maps


def _assemble(results):
    host = _CACHE["host"]
    # scatter the col-sum contributions into global per-sample partials
    den_col = np.zeros(_B, dtype=np.float32)
    for c in range(_M):
        o2 = np.asarray(results[c]["out2"], dtype=np.float32)
        for qi, b in enumerate(host[c]["blocks"]):
            den_col[b * _BL:(b + 1) * _BL] += o2[qi]
    total = 0.0
    for c in range(_M):
        o = np.asarray(results[c]["out"], dtype=np.float32)
        h = host[c]
        den = o[:, 0::4] + o[:, 1::4] + o[:, 2::4] + o[:, 3::4]  # [128, RT]
        R = c * _BL
        den = den + den_col[R:R + _BL].reshape(_RT, 128).T + h["corr"]
        per = np.log(den) - h["pos"]
        total += float(per.sum())
    return np.float32(total / _B)


def _run(inputs, trace=False, **trace_kwargs):
    from concourse.bass_utils import run_bass_kernel_spmd
    nc = _get_nc()
    in_maps = _prep_inputs(**inputs)
    res = run_bass_kernel_spmd(nc, in_maps, core_ids=list(range(_M)),
                               trace=trace, **trace_kwargs)
    return _assemble(res.results), res


def kernel(centers, features, targets):
    out, _ = _run({"centers": centers, "features": features, "targets": targets})
    return out
